# revision 13
# baseline (speedup 1.0000x reference)
"""Trainium2 Bass kernel for nn_CustomMHA (sparse head-gathered MHA).

Math (reference):
  q/k/v projections of x (all heads) and x1/x2/x3 (one head, head_idx),
  heads gathered: out head i = special (x1/x2/x3 @ W[head_idx]) when
  permutation[i]==0 else x @ W[head_idxs[permutation[i]-1]]; full T x T
  softmax attention per head; concat heads; out = y @ W_proj.T + b_proj.

Strategy (8 NeuronCores, SPMD, single launch, no collectives):
  - Head dedup: output positions that gather the same source head produce
    identical attention output, so their c_proj columns are summed and the
    attention runs once per DISTINCT source head.  With U distinct normal
    heads, core c owns nh = ceil(U/8) heads and computes their q/k/v for
    all tokens, full T x T attention for both batches, and a partial
    c_proj over all tokens (host sums partials).
  - Special (x1/x2/x3) head: each core projects the special q for its own
    512-token chunk and the special k/v for its chunk's batch (from
    x2/x3), runs attention for its 512 queries, and emits the special
    c_proj into a separate small output OUTS (keeps the SPMD instruction
    stream uniform); the host adds OUTS into the right rows.
  - All streams are bf16 (PE runs bf16 at fp32r speed; halves DMA/SBUF),
    accumulation stays fp32 in PSUM.  v is projected in natural
    [token, D] orientation (matmul cost is free-size only), feeding the
    [v | ones] y-matmul operand without PE transposes; the ones column
    provides the softmax denominator for free.
  - c_proj runs in natural [token, C] orientation; partial outputs are
    staged bf16 and summed on the host in fp32.
"""

import numpy as np
import ml_dtypes
from contextlib import ExitStack

import concourse.bass as bass
import concourse.tile as tile
from concourse import bacc, mybir
from concourse.bass_utils import run_bass_kernel_spmd

F32 = mybir.dt.float32
BF16 = mybir.dt.bfloat16
AF = mybir.ActivationFunctionType
BF = ml_dtypes.bfloat16

B, T, C, H, D = 2, 2048, 1024, 16, 64
NC = 8
NT = B * T            # 4096 tokens total, flat index = b*T + t
P = 128
NCT = C // P          # 8 contraction tiles
TCH = 512             # token chunk (projection / per-core special queries)
NTCH = NT // TCH      # 8
QCH = 1024            # attention query chunk
NKT = T // P          # 16 key tiles per batch
SCALE = 1.0 / np.sqrt(D)


def _build(nh):
    """Single SPMD launch; nh = normal heads per core."""
    nc = bacc.Bacc("TRN2", target_bir_lowering=False, debug=False, num_devices=NC)

    def din(name, shape, dt=BF16):
        return nc.dram_tensor(name, shape, dt, kind="ExternalInput").ap()

    XT = din("XT", [C, NT])              # x.T
    X1C = din("X1C", [C, TCH])           # x1.T own-chunk columns
    X2B = din("X2B", [C, T])             # x2.T own-batch columns
    X3B = din("X3B", [C, T])             # x3.T own-batch columns
    WQK = din("WQK", [C, nh * P])        # per head: [Wq_h.T | Wk_h.T]
    WV = din("WV", [C, nh * D])          # per head: Wv_h.T
    WSQ = din("WSQ", [C, D])             # special head weight slices (.T)
    WSK = din("WSK", [C, D])
    WSV = din("WSV", [C, D])
    WPN = din("WPN", [nh * D, C])        # summed W_proj cols per head (.T)
    WPS = din("WPS", [D, C])             # summed special W_proj cols (.T)
    BQK = din("BQK", [nh * P, 1], F32)   # per-partition q/k biases
    BSQK = din("BSQK", [D, 2], F32)      # special q/k per-partition biases
    BVB = din("BVB", [P, (nh + 1) * D], F32)  # broadcast v-bias tiles

    OUTN = nc.dram_tensor("OUTN", [NT, C], BF16, kind="ExternalOutput").ap()
    OUTS = nc.dram_tensor("OUTS", [TCH, C], BF16, kind="ExternalOutput").ap()

    with tile.TileContext(nc) as tc, ExitStack() as ctx:
        consts = ctx.enter_context(tc.tile_pool(name="consts", bufs=1))
        big = ctx.enter_context(tc.tile_pool(name="big", bufs=1))
        xp = ctx.enter_context(tc.tile_pool(name="xp", bufs=5))
        xsp = ctx.enter_context(tc.tile_pool(name="xsp", bufs=3))
        expp = ctx.enter_context(tc.tile_pool(name="expp", bufs=7))
        stg = ctx.enter_context(tc.tile_pool(name="stg", bufs=6))
        ysp = ctx.enter_context(tc.tile_pool(name="ysp", bufs=4))
        rcp = ctx.enter_context(tc.tile_pool(name="rcp", bufs=2))
        # PSUM: pp 2x[128,512]f32 = 2 banks, pa 2x[128,1024] = 4, py 1x = 2
        pp = ctx.enter_context(tc.tile_pool(name="pp", bufs=2, space="PSUM"))
        pa = ctx.enter_context(tc.tile_pool(name="pa", bufs=2, space="PSUM"))
        py_ = ctx.enter_context(tc.tile_pool(name="py", bufs=1, space="PSUM"))

        # ---- constants
        def load_w(ap_dram, m, engine):
            # [C, m] dram -> [128, NCT*m] sbuf, block ci = contraction tile
            t = consts.tile([P, NCT * m], BF16, tag=f"w{ap_dram.tensor.name}")
            engine.dma_start(
                t[:].rearrange("p (a m) -> p a m", a=NCT),
                ap_dram.rearrange("(a p) m -> p a m", p=P),
            )
            return t

        wqk = load_w(WQK, nh * P, nc.scalar)
        wv = load_w(WV, nh * D, nc.scalar)
        wsq = load_w(WSQ, D, nc.scalar)
        wsk = load_w(WSK, D, nc.scalar)
        wsv = load_w(WSV, D, nc.scalar)
        wpn = consts.tile([nh * D, C], BF16, tag="wpn")
        nc.gpsimd.dma_start(wpn[:], WPN[:])
        wps = consts.tile([D, C], BF16, tag="wps")
        nc.gpsimd.dma_start(wps[:], WPS[:])
        bqk = consts.tile([nh * P, 1], F32, tag="bqk")
        nc.gpsimd.dma_start(bqk[:], BQK[:])
        bsqk = consts.tile([D, 2], F32, tag="bsqk")
        nc.gpsimd.dma_start(bsqk[:], BSQK[:])
        bvb = consts.tile([P, (nh + 1) * D], F32, tag="bvb")
        nc.gpsimd.dma_start(bvb[:], BVB[:])

        qkT = [big.tile([P, NT], BF16, tag=f"qkT{h}", name=f"qkT{h}")
               for h in range(nh)]
        # k half of the packed projection, DMA-shifted to base partition 0
        # (PE matmul requires lhsT and rhs at the same base partition)
        kTs = [big.tile([D, NT], BF16, tag=f"kTs{h}", name=f"kTs{h}")
               for h in range(nh)]
        vaug = [big.tile([P, 65 * B * NKT], BF16, tag=f"vaug{h}",
                         name=f"vaug{h}") for h in range(nh)]
        yT = big.tile([P, NT], BF16, tag="yT")    # rows h*64:(h+1)*64 per head
        skT = big.tile([D, T], BF16, tag="skT")
        sqT = big.tile([D, TCH], BF16, tag="sqT")
        vaugS = big.tile([P, 65 * NKT], BF16, tag="vaugS")
        ySN = big.tile([D, TCH], BF16, tag="ySN")

        # softmax-denominator ones columns (col 64 of each 65-wide group)
        for h in range(nh):
            nc.gpsimd.memset(
                vaug[h][:].rearrange("p (a m) -> p a m", m=65)[:, :, 64:65], 1.0
            )
        nc.gpsimd.memset(
            vaugS[:].rearrange("p (a m) -> p a m", m=65)[:, :, 64:65], 1.0
        )

        # ---- projections (q/k packed transposed; v natural) per 512 chunk
        def phase_p(tj):
            sl = bass.ts(tj, TCH)
            xt = xp.tile([P, NCT * TCH], BF16, tag="xt")
            xtv = xt[:].rearrange("p (a m) -> p a m", a=NCT)
            xdr = XT[:, sl].rearrange("(a p) m -> p a m", p=P)
            if tj == 0:
                # per-ci pieces so the first matmul starts ~8x sooner
                for ci in range(NCT):
                    nc.sync.dma_start(xtv[:, ci, :], xdr[:, ci, :])
            else:
                nc.sync.dma_start(xtv, xdr)
            for h in range(nh):
                ps = pp.tile([P, TCH], F32, tag="proj")
                for ci in range(NCT):
                    nc.tensor.matmul(
                        ps[:],
                        wqk[:, ci * nh * P + h * P: ci * nh * P + (h + 1) * P],
                        xtv[:, ci, :],
                        start=(ci == 0), stop=(ci == NCT - 1),
                    )
                with nc.allow_low_precision(reason="bf16 store"):
                    nc.vector.tensor_scalar_add(
                        qkT[h][:, sl], ps[:], bqk[h * P:(h + 1) * P, 0:1]
                    )
                nc.gpsimd.dma_start(kTs[h][0:D, sl], qkT[h][D:2 * D, sl])
                for w in range(4):
                    pv = pp.tile([P, D], F32, tag="proj")
                    for ci in range(NCT):
                        nc.tensor.matmul(
                            pv[:],
                            xtv[:, ci, w * P:(w + 1) * P],
                            wv[:, ci * nh * D + h * D: ci * nh * D + (h + 1) * D],
                            start=(ci == 0), stop=(ci == NCT - 1),
                        )
                    base = 65 * (tj * 4 + w)
                    with nc.allow_low_precision(reason="bf16 store"):
                        nc.vector.tensor_add(
                            vaug[h][:, base:base + D], pv[:],
                            bvb[:, h * D:(h + 1) * D],
                        )

        # ---- special projections: sq (own chunk), skT (own batch), sv nat
        def phase_s():
            xc = xsp.tile([P, NCT * TCH], BF16, tag="xs")
            xcv = xc[:].rearrange("p (a m) -> p a m", a=NCT)
            nc.scalar.dma_start(xcv, X1C.rearrange("(a p) m -> p a m", p=P))
            ps = pp.tile([P, TCH], F32, tag="proj")
            for ci in range(NCT):
                nc.tensor.matmul(
                    ps[0:D, :], wsq[:, ci * D:(ci + 1) * D], xcv[:, ci, :],
                    start=(ci == 0), stop=(ci == NCT - 1),
                )
            with nc.allow_low_precision(reason="bf16 store"):
                nc.vector.tensor_scalar_add(sqT[:], ps[0:D, :], bsqk[:, 0:1])
            for cch in range(4):
                x2 = xsp.tile([P, NCT * TCH], BF16, tag="xs")
                x2v = x2[:].rearrange("p (a m) -> p a m", a=NCT)
                nc.scalar.dma_start(
                    x2v,
                    X2B[:, cch * TCH:(cch + 1) * TCH].rearrange(
                        "(a p) m -> p a m", p=P),
                )
                ps2 = pp.tile([P, TCH], F32, tag="proj")
                for ci in range(NCT):
                    nc.tensor.matmul(
                        ps2[0:D, :], wsk[:, ci * D:(ci + 1) * D], x2v[:, ci, :],
                        start=(ci == 0), stop=(ci == NCT - 1),
                    )
                with nc.allow_low_precision(reason="bf16 store"):
                    nc.vector.tensor_scalar_add(
                        skT[:, cch * TCH:(cch + 1) * TCH], ps2[0:D, :],
                        bsqk[:, 1:2],
                    )
            for cch in range(4):
                x3 = xsp.tile([P, NCT * TCH], BF16, tag="xs")
                x3v = x3[:].rearrange("p (a m) -> p a m", a=NCT)
                nc.scalar.dma_start(
                    x3v,
                    X3B[:, cch * TCH:(cch + 1) * TCH].rearrange(
                        "(a p) m -> p a m", p=P),
                )
                for w in range(4):
                    pv = pp.tile([P, D], F32, tag="proj")
                    for ci in range(NCT):
                        nc.tensor.matmul(
                            pv[:],
                            x3v[:, ci, w * P:(w + 1) * P],
                            wsv[:, ci * D:(ci + 1) * D],
                            start=(ci == 0), stop=(ci == NCT - 1),
                        )
                    base = 65 * (cch * 4 + w)
                    with nc.allow_low_precision(reason="bf16 store"):
                        nc.vector.tensor_add(
                            vaugS[:, base:base + D], pv[:],
                            bvb[:, nh * D:(nh + 1) * D],
                        )

        # ---- softmax-normalize a y psum into a bf16 destination
        def normalize(yp, w, dst, shift=False):
            ys = ysp.tile([65, QCH], F32, tag="ystage")
            nc.vector.tensor_copy(ys[:, 0:w], yp[0:65, 0:w])
            # partition_broadcast reads the tile's physical partition 0, so
            # DMA-shift the sum row to a partition-0 tile first.
            srow = rcp.tile([1, QCH], F32, tag="srow")
            nc.gpsimd.dma_start(srow[0:1, 0:w], ys[64:65, 0:w])
            rc = rcp.tile([1, QCH], F32, tag="rc")
            with nc.allow_low_precision(reason="fp32 bits"):
                nc.vector.reciprocal(rc[0:1, 0:w], srow[0:1, 0:w])
            bc = rcp.tile([D, QCH], F32, tag="bc")
            nc.gpsimd.partition_broadcast(bc[0:D, 0:w], rc[0:1, 0:w])
            if shift:
                # dst is not at base partition 0: bounce through a tile
                ys2 = ysp.tile([D, QCH], BF16, tag="yshift")
                with nc.allow_low_precision(reason="bf16 store"):
                    nc.vector.tensor_mul(ys2[0:D, 0:w], ys[0:D, 0:w],
                                         bc[0:D, 0:w])
                nc.gpsimd.dma_start(dst, ys2[0:D, 0:w])
            else:
                with nc.allow_low_precision(reason="bf16 store"):
                    nc.vector.tensor_mul(dst, ys[0:D, 0:w], bc[0:D, 0:w])

        # ---- normal attention for one (head-slot, batch, query chunk)
        def phase_a(h, b, qs0):
            qs = b * T + qs0
            yp = py_.tile([P, QCH], F32, tag="y")
            exs = []
            for k in range(NKT):
                ap_ = pa.tile([P, QCH], F32, tag="att")
                for hf in range(2):
                    nc.tensor.matmul(
                        ap_[:, hf * 512:(hf + 1) * 512],
                        kTs[h][0:D, b * T + k * P: b * T + (k + 1) * P],
                        qkT[h][0:D, qs + hf * 512: qs + (hf + 1) * 512],
                        start=True, stop=True,
                    )
                ex = expp.tile([P, QCH], BF16, tag="exp")
                nc.scalar.activation(ex[:], ap_[:], AF.Exp, scale=float(SCALE))
                exs.append(ex)
            for k in range(NKT):
                base = 65 * (b * NKT + k)
                for hf in range(2):
                    nc.tensor.matmul(
                        yp[0:65, hf * 512:(hf + 1) * 512],
                        vaug[h][:, base:base + 65],
                        exs[k][:, hf * 512:(hf + 1) * 512],
                        start=(k == 0), stop=(k == NKT - 1),
                    )
            normalize(yp, QCH, yT[h * D:(h + 1) * D, qs:qs + QCH], shift=(h > 0))

        # ---- special attention: own 512 queries over own batch's keys
        def phase_sa():
            yp = py_.tile([P, QCH], F32, tag="y")
            exs = []
            for k in range(NKT):
                ap_ = pa.tile([P, QCH], F32, tag="att")
                nc.tensor.matmul(
                    ap_[:, 0:TCH], skT[:, k * P:(k + 1) * P], sqT[:],
                    start=True, stop=True,
                )
                ex = expp.tile([P, QCH], BF16, tag="exp")
                nc.scalar.activation(
                    ex[:, 0:TCH], ap_[:, 0:TCH], AF.Exp, scale=float(SCALE)
                )
                exs.append(ex)
            for k in range(NKT):
                nc.tensor.matmul(
                    yp[0:65, 0:TCH], vaugS[:, 65 * k:65 * k + 65],
                    exs[k][:, 0:TCH],
                    start=(k == 0), stop=(k == NKT - 1),
                )
            normalize(yp, TCH, ySN[0:D, 0:TCH])

        # ---- partial c_proj -> OUTN for one 128-token tile
        def phase_c(tt, ei):
            st = stg.tile([P, QCH], BF16, tag="stage")
            for hf in range(2):
                cp = pp.tile([P, TCH], F32, tag="proj")
                nc.tensor.matmul(
                    cp[:], yT[0:nh * D, tt * P:(tt + 1) * P],
                    wpn[:, hf * 512:(hf + 1) * 512],
                    start=True, stop=True,
                )
                if (2 * tt + hf + ei) % 3 == 1:
                    nc.scalar.copy(st[:, hf * 512:(hf + 1) * 512], cp[:])
                else:
                    with nc.allow_low_precision(reason="bf16 store"):
                        nc.vector.tensor_copy(
                            st[:, hf * 512:(hf + 1) * 512], cp[:])
            nc.sync.dma_start(OUTN[tt * P:(tt + 1) * P, :], st[:])

        # ---- special c_proj -> OUTS for one 128-token tile of own chunk
        def phase_cs(w):
            st = stg.tile([P, QCH], BF16, tag="stage")
            for hf in range(2):
                cp = pp.tile([P, TCH], F32, tag="proj")
                nc.tensor.matmul(
                    cp[:], ySN[0:D, w * P:(w + 1) * P],
                    wps[:, hf * 512:(hf + 1) * 512],
                    start=True, stop=True,
                )
                with nc.allow_low_precision(reason="bf16 store"):
                    nc.vector.tensor_copy(st[:, hf * 512:(hf + 1) * 512], cp[:])
            nc.sync.dma_start(OUTS[w * P:(w + 1) * P, :], st[:])

        # Emission order = scheduler priority.  The special path runs FIRST:
        # its projections are ready before the big x.T stream lands, and its
        # ACT-gated attention chain fills the scalar engine's head start
        # while the normal projections run on PE; c_proj fills PE gaps.
        phase_s()
        for tj in range(4):
            phase_p(tj)
        phase_sa()
        for h in range(nh):
            phase_a(h, 0, 0)
            phase_a(h, 0, QCH)
        for tj in range(4, 8):
            phase_p(tj)
        for w in range(4):
            phase_cs(w)
        for h in range(nh):
            phase_a(h, 1, 0)
        for tt in range(16):
            phase_c(tt, 0)
        for h in range(nh):
            phase_a(h, 1, QCH)
        for tt in range(16, 32):
            phase_c(tt, 1)

    nc.compile()
    return nc


_CACHE = {}


def _get_nc(nh):
    if nh not in _CACHE:
        _CACHE[nh] = _build(nh)
    return _CACHE[nh]


def kernel(x1, x2, x3, x, W_attn, b_attn, W_proj, b_proj, head_idx, head_idxs,
           permutation):
    f32 = np.float32
    x1 = np.asarray(x1, f32).reshape(NT, C)
    x2 = np.asarray(x2, f32).reshape(NT, C)
    x3 = np.asarray(x3, f32).reshape(NT, C)
    x = np.asarray(x, f32).reshape(NT, C)
    W_attn = np.asarray(W_attn, f32)
    b_attn = np.asarray(b_attn, f32)
    W_proj = np.asarray(W_proj, f32)
    b_proj = np.asarray(b_proj, f32)
    hidx = int(head_idx)
    head_idxs = np.asarray(head_idxs).astype(np.int64)
    perm = np.asarray(permutation).astype(np.int64)

    Wq, Wk, Wv = W_attn[:C], W_attn[C:2 * C], W_attn[2 * C:]
    bq, bk, bv = b_attn[:C], b_attn[C:2 * C], b_attn[2 * C:]

    # output head position -> source head (special = x1/x2/x3 path)
    special_pos = [i for i in range(H) if perm[i] == 0]
    normal_pairs = [(i, int(head_idxs[perm[i] - 1]))
                    for i in range(H) if perm[i] != 0]
    srcs = sorted(set(s for _, s in normal_pairs))
    U = len(srcs)
    nh = max(1, (U + NC - 1) // NC)

    xT = np.ascontiguousarray(x.T.astype(BF))
    x1T = x1.T
    x2Tb = [np.ascontiguousarray(x2.T[:, b * T:(b + 1) * T].astype(BF))
            for b in range(B)]
    x3Tb = [np.ascontiguousarray(x3.T[:, b * T:(b + 1) * T].astype(BF))
            for b in range(B)]

    hss = slice(hidx * D, (hidx + 1) * D)
    wsq = np.ascontiguousarray(Wq[hss].T.astype(BF))
    wsk = np.ascontiguousarray(Wk[hss].T.astype(BF))
    wsv = np.ascontiguousarray(Wv[hss].T.astype(BF))
    wps = np.zeros((C, D), f32)
    for i in special_pos:
        wps += W_proj[:, i * D:(i + 1) * D]
    wpsT = np.ascontiguousarray(wps.T.astype(BF))
    bsqk = np.stack([bq[hss], bk[hss]], axis=1).astype(f32)

    in_maps = []
    for c in range(NC):
        wqk2 = np.zeros((C, nh * P), f32)
        wv2 = np.zeros((C, nh * D), f32)
        wpn2 = np.zeros((nh * D, C), f32)
        bqk2 = np.zeros((nh * P, 1), f32)
        bvb2 = np.zeros((P, (nh + 1) * D), f32)
        for j in range(nh):
            u = c * nh + j
            if u >= U:
                continue
            hsrc = srcs[u]
            hs = slice(hsrc * D, (hsrc + 1) * D)
            wqk2[:, j * P:j * P + D] = Wq[hs].T
            wqk2[:, j * P + D:(j + 1) * P] = Wk[hs].T
            wv2[:, j * D:(j + 1) * D] = Wv[hs].T
            wpsum = np.zeros((C, D), f32)
            for i, s in normal_pairs:
                if s == hsrc:
                    wpsum += W_proj[:, i * D:(i + 1) * D]
            wpn2[j * D:(j + 1) * D, :] = wpsum.T
            bqk2[j * P:j * P + D, 0] = bq[hs]
            bqk2[j * P + D:(j + 1) * P, 0] = bk[hs]
            bvb2[:, j * D:(j + 1) * D] = bv[hs][None, :]
        bvb2[:, nh * D:(nh + 1) * D] = bv[hss][None, :]
        bc_ = c // (NC // B)  # batch of this core's 512-token chunk
        in_maps.append({
            "XT": xT,
            "X1C": np.ascontiguousarray(
                x1T[:, c * TCH:(c + 1) * TCH].astype(BF)),
            "X2B": x2Tb[bc_],
            "X3B": x3Tb[bc_],
            "WQK": wqk2.astype(BF), "WV": wv2.astype(BF),
            "WSQ": wsq, "WSK": wsk, "WSV": wsv,
            "WPN": wpn2.astype(BF), "WPS": wpsT,
            "BQK": bqk2, "BSQK": bsqk, "BVB": bvb2,
        })

    ncA = _get_nc(nh)
    res = run_bass_kernel_spmd(ncA, in_maps, list(range(NC))).results

    out = np.zeros((NT, C), f32)
    for c in range(NC):
        out += res[c]["OUTN"].astype(f32)
    for c in range(NC):
        out[c * TCH:(c + 1) * TCH] += res[c]["OUTS"].astype(f32)
    out += b_proj[None, :]
    return out.reshape(B, T, C).astype(f32)
